# revision 2
# baseline (speedup 1.0000x reference)
"""Masked-attention kernel for AWS Trainium2, 8-core SPMD (Bass/Tile).

Problem: B=4, S=4096, E=512, A=64 masked attention
    out[b,q,a] = softmax_k(mask(qp @ kp^T)/sqrt(A)) @ vp,   *p = x @ w*

Sharding (data-parallel, no collectives): core c -> (batch b=c//2, query half
h=c%2).  Host-side prep is layout-only (transposes, mask invert + uint8 view,
mask pre-tiling to the device tile layout, fp16 re-encoding of the binary
mask for the half that rides the HWDGE ring).

Design (evolved from a 160-168us all-SWDGE fp16 baseline; ~164us measured):
  - q/k/v/w loaded RAW f32 over the sync-engine HWDGE ring, priority-ordered
    by consumption deadline; the first k/q/v groups are split into 1MB halves
    so the PE warms up ~15us sooner.  Projections consume the raw chunks
    directly as float32r matmuls (1 cycle/row at N=512 - no cast anywhere).
  - The scalar-engine queue carries only the 3 tiny weight DMAs + the exp
    stream + output stores: a large DMA_DIRECT2D on that queue would block
    every exp behind its descriptor enqueue (measured 14-19us stalls).
  - Masks: u8 HBM data is host-pre-tiled so each [128,4,QW] group is one
    contiguous 512KB SWDGE cast-DMA.  qc0 groups + even qc1 groups go via
    SWDGE (u8->fp16 in flight); odd qc1 groups ride the sync ring as
    host-encoded fp16, so neither stream alone has to match the main-loop
    mask consumption rate.
  - vp is computed as vpT = wv.T @ v (f32r), then PE-transposed per 128-key
    tile via an on-device identity (the XBAR DMA transpose corrupts even
    partitions when pipelined), into vp_all [key, A | ones] fp16; the ones
    column makes the softmax denominator fall out of the attn@V matmul as
    output row 64 - no partition reductions anywhere.
  - Main loop is software-pipelined with skew 2: scores(kt+2) are emitted
    before exp/mask-mult/attnV(kt), so the in-order PE queue never parks on
    attnV(i) (which waits on the DVE mask-mult) while later scores sit
    behind it.  Triple-buffered score PSUM (psS bufs=3) + single outT bank
    pair (psO bufs=1, qc halves processed sequentially with the qc0
    normalize overlapping the qc1 loop) fit exactly in 8 PSUM banks.
    The steady-state loop runs at ~1.08us/iter, ACT(exp)-bound, with the
    PE HAM clock staying warm.
  - 1/sqrt(A) is applied via the exp's ACT scale operand; the softmax needs
    no max-subtraction (scores ~ N(0,1) after scaling); normalization is a
    deferred reciprocal broadcast via gpsimd partition_broadcast.

Measured on 8 axon-attached TRN2 NeuronCores: ~164 us HW exec, rel err
(L2) ~5e-4 vs the f32 reference.
"""

import os
import sys

import numpy as np

_TRN_REPO_PATHS = ["/opt/trn_rl_repo", "/root/.axon_site", "/root/.axon_site/_ro/trn_rl_repo"]
for _p in _TRN_REPO_PATHS:
    if os.path.isdir(_p) and _p not in sys.path:
        sys.path.append(_p)
os.environ.setdefault("MYCRO_LOCAL_CACHE", "1")

B, S, E, A = 4, 4096, 512, 64
QL = 2048          # queries per core
EC = E // 128      # contraction chunks
KT = S // 128      # key tiles
QC = 2             # query chunks
QW = QL // QC      # query-chunk width (1024)
KG = S // 1024     # key groups of 1024 (4)
QG = QL // 1024    # query groups of 1024 (2)
N_CORES = 8

_NC_CACHE = {}


def _build():
    import concourse.bass as bass
    import concourse.mybir as mybir
    import concourse.tile as tile
    from concourse import bacc
    from concourse import masks

    F32 = mybir.dt.float32
    F32R = mybir.dt.float32r
    F16 = mybir.dt.float16
    F8 = mybir.dt.float8e4
    U8 = mybir.dt.uint8
    Exp = mybir.ActivationFunctionType.Exp
    MULT = mybir.AluOpType.mult
    DR = mybir.MatmulPerfMode.DoubleRow

    nc = bacc.Bacc("TRN2", target_bir_lowering=False, debug=False, num_devices=N_CORES)
    # all raw inputs declared float32r (numpy-identical to f32) so HWDGE
    # dtype checks pass and matmuls take the fast f32r path directly
    qT = nc.dram_tensor("qT", [E, QL], F32R, kind="ExternalInput")
    kT = nc.dram_tensor("kT", [E, S], F32R, kind="ExternalInput")
    vT = nc.dram_tensor("vT", [E, S], F32R, kind="ExternalInput")
    mbT = nc.dram_tensor("mbT", [QC, 8, 128, 4, QW], U8, kind="ExternalInput")
    mb16 = nc.dram_tensor("mb16", [8, 128, 4, QW], F16, kind="ExternalInput")
    wq = nc.dram_tensor("wq", [E, A], F32R, kind="ExternalInput")
    wk = nc.dram_tensor("wk", [E, A], F32R, kind="ExternalInput")
    wv = nc.dram_tensor("wv", [E, A], F32R, kind="ExternalInput")
    out = nc.dram_tensor("out", [A, QL], F32, kind="ExternalOutput")

    with tile.TileContext(nc) as tc:
        with (
            tc.tile_pool(name="persist", bufs=1) as pp,
            tc.tile_pool(name="kraw", bufs=2) as krp,
            tc.tile_pool(name="qraw", bufs=1) as qrp,
            tc.tile_pool(name="halfp", bufs=4) as hp,
            tc.tile_pool(name="vraw", bufs=2) as vrp,
            tc.tile_pool(name="loop", bufs=2) as lp,
            tc.tile_pool(name="trsp", bufs=2) as trp,
            tc.tile_pool(name="maskp", bufs=4) as mp,
            tc.tile_pool(name="psS", bufs=3, space=bass.MemorySpace.PSUM) as psS,
            tc.tile_pool(name="psO", bufs=1, space=bass.MemorySpace.PSUM) as psO,
        ):
            # ---- weights (raw f32r, ACT ring) + exp-table prewarm ----
            wq_sb = pp.tile([128, EC, A], F32R, tag="wq")
            wk_sb = pp.tile([128, EC, A], F32R, tag="wk")
            wv_sb = pp.tile([128, EC, A], F32R, tag="wv")
            nc.scalar.dma_start(out=wq_sb[:, :, :], in_=wq.ap().rearrange("(c p) a -> p c a", p=128))
            nc.scalar.dma_start(out=wk_sb[:, :, :], in_=wk.ap().rearrange("(c p) a -> p c a", p=128))
            nc.scalar.dma_start(out=wv_sb[:, :, :], in_=wv.ap().rearrange("(c p) a -> p c a", p=128))
            ident = pp.tile([A, A], F16, tag="ident")
            masks.make_identity(nc, ident[:, :])
            warm = pp.tile([1, 8], F32, tag="warm")
            nc.vector.memset(warm[:, :], 0.0)
            nc.scalar.activation(warm[:, :], warm[:, :], Exp)

            # ---- persistent projection outputs ----
            kpT = pp.tile([A, S], F16, tag="kpT")
            qpT = pp.tile([A, QL], F16, tag="qpT")
            vp_all = pp.tile([128, KT, A + 1], F16, tag="vpall")
            nc.vector.memset(vp_all[:, :, A:A + 1], 1.0)

            # ---- input DMAs ----
            # SP ring (nc.sync): kT, vT loads + vp transposes + output stores
            # ACT ring (nc.scalar): weights + qT loads
            # SWDGE (nc.gpsimd): mask u8->fp16 cast, in consumption order
            def k_dma(g):
                t = krp.tile([128, EC, 1024], F32R, tag="kraw")
                nc.sync.dma_start(
                    out=t[:, :, :],
                    in_=kT[:, g * 1024:(g + 1) * 1024].rearrange("(c p) n -> p c n", p=128),
                )
                return t

            def half_dma(src_t, col0, eng=None):
                t = hp.tile([128, EC, 512], F32R, tag="half")
                (eng or nc.sync).dma_start(
                    out=t[:, :, :],
                    in_=src_t[:, col0:col0 + 512].rearrange("(c p) n -> p c n", p=128),
                )
                return t

            def proj_sub(w_sb, half, dstT, col0):
                ps = psS.tile([A, 512], F32, tag="psS")
                for ec in range(EC):
                    nc.tensor.matmul(
                        ps[:, :], w_sb[:, ec, :], half[:, ec, :],
                        start=(ec == 0), stop=(ec == EC - 1),
                    )
                nc.vector.tensor_copy(dstT[:, col0:col0 + 512], ps[:, :])

            def vp_sub(half, j0):
                ps = psS.tile([A, 512], F32, tag="psS")
                for ec in range(EC):
                    nc.tensor.matmul(
                        ps[:, :], wv_sb[:, ec, :], half[:, ec, :],
                        start=(ec == 0), stop=(ec == EC - 1),
                    )
                for jj in range(4):
                    ti = trp.tile([A, 128], F16, tag="ti")
                    nc.vector.tensor_copy(ti[:, :], ps[:, jj * 128:(jj + 1) * 128])
                    tp = psS.tile([128, A], F16, tag="psS")
                    nc.tensor.transpose(tp[:, :], ti[:, :], ident[:, :])
                    nc.vector.tensor_copy(vp_all[:, j0 + jj, 0:A], tp[:, :])

            def v_dma(g):
                t = vrp.tile([128, EC, 1024], F32R, tag="vraw")
                nc.sync.dma_start(
                    out=t[:, :, :],
                    in_=vT[:, g * 1024:(g + 1) * 1024].rearrange("(c p) n -> p c n", p=128),
                )
                return t

            def q_dma(qg):
                t = qrp.tile([128, EC, 1024], F32R, tag="qraw")
                nc.sync.dma_start(
                    out=t[:, :, :],
                    in_=qT[:, qg * 1024:(qg + 1) * 1024].rearrange("(c p) n -> p c n", p=128),
                )
                return t

            mask_tiles = {}

            def mask_group(qc, g4):
                if (qc, g4) not in mask_tiles:
                    mb = mp.tile([128, 4, QW], F16, tag="mask")
                    if qc == 0 or g4 % 2 == 0:
                        nc.gpsimd.dma_start(out=mb[:, :, :], in_=mbT[qc, g4, :, :, :])
                    else:
                        # alternate qc1 masks onto the sync ring (host fp16) so
                        # SWDGE and the ring each carry half the qc1 demand
                        nc.sync.dma_start(out=mb[:, :, :], in_=mb16[g4, :, :, :])
                    mask_tiles[(qc, g4)] = mb
                return mask_tiles[(qc, g4)]

            # ---- projection groups (consume one raw 1024-col group each) ----
            def kp_group(g, kraw):
                kp_ps = psS.tile([A, 1024], F32, tag="psS")
                for nn in range(2):
                    for ec in range(EC):
                        nc.tensor.matmul(
                            kp_ps[:, nn * 512:(nn + 1) * 512],
                            wk_sb[:, ec, :],
                            kraw[:, ec, nn * 512:(nn + 1) * 512],
                            start=(ec == 0), stop=(ec == EC - 1),
                        )
                nc.vector.tensor_copy(kpT[:, g * 1024:(g + 1) * 1024], kp_ps[:, :])

            def qp_group(qg, qraw):
                qp_ps = psS.tile([A, 1024], F32, tag="psS")
                for nn in range(2):
                    for ec in range(EC):
                        nc.tensor.matmul(
                            qp_ps[:, nn * 512:(nn + 1) * 512],
                            wq_sb[:, ec, :],
                            qraw[:, ec, nn * 512:(nn + 1) * 512],
                            start=(ec == 0), stop=(ec == EC - 1),
                        )
                nc.vector.tensor_copy(qpT[:, qg * 1024:(qg + 1) * 1024], qp_ps[:, :])

            def vp_group(g, vraw):
                vp_ps = psS.tile([A, 1024], F32, tag="psS")
                for nn in range(2):
                    for ec in range(EC):
                        nc.tensor.matmul(
                            vp_ps[:, nn * 512:(nn + 1) * 512],
                            wv_sb[:, ec, :],
                            vraw[:, ec, nn * 512:(nn + 1) * 512],
                            start=(ec == 0), stop=(ec == EC - 1),
                        )
                # PE transpose [A, 128] -> [128, A] per key tile (the XBAR
                # DMA transpose corrupts even partitions when pipelined here).
                for j in range(8):
                    ti = trp.tile([A, 128], F16, tag="ti")
                    nc.vector.tensor_copy(ti[:, :], vp_ps[:, j * 128:(j + 1) * 128])
                    tp = psS.tile([128, A], F16, tag="psS")
                    nc.tensor.transpose(tp[:, :], ti[:, :], ident[:, :])
                    nc.vector.tensor_copy(vp_all[:, g * 8 + j, 0:A], tp[:, :])

            # ---- main attention tile step (software-pipelined, skew 2) ----
            # scores(kt) are emitted 2 iterations ahead of their exp/mult/
            # attnV tail so the in-order PE queue never parks on attnV(i)
            # waiting for mult(i) while later scores sit behind it.
            outTs = {}
            sps = {}

            def emit_scores(qc, kt):
                s_ps = psS.tile([128, QW], F32, tag="psS")
                for qn in range(QW // 512):
                    nc.tensor.matmul(
                        s_ps[:, qn * 512:(qn + 1) * 512],
                        kpT[:, kt * 128:(kt + 1) * 128],
                        qpT[:, qc * QW + qn * 512: qc * QW + (qn + 1) * 512],
                        start=True, stop=True,
                    )
                sps[(qc, kt)] = s_ps

            def emit_tail(qc, kt):
                mask_bf = mask_group(qc, kt // 4)
                s_ps = sps.pop((qc, kt))
                e_sb = lp.tile([128, QW], F16, tag="exp")
                nc.scalar.activation(e_sb[:, :], s_ps[:, :], Exp, scale=1.0 / np.sqrt(A))
                attn = lp.tile([128, QW], F16, tag="attn")
                nc.vector.tensor_tensor(attn[:, :], e_sb[:, :], mask_bf[:, kt % 4, :], MULT)
                for qn in range(QW // 512):
                    nc.tensor.matmul(
                        outTs[qc][:, qn * 512:(qn + 1) * 512],
                        vp_all[:, kt, :],
                        attn[:, qn * 512:(qn + 1) * 512],
                        start=(kt == 0), stop=(kt == KT - 1),
                    )

            def attn_step(qc, kt):
                emit_scores(qc, kt)
                if kt >= 2:
                    emit_tail(qc, kt - 2)

            # ---- emission schedule ----
            # sync ring carries ALL big loads, priority-ordered by deadline;
            # WAR waits on raw bufs pace later loads behind consumption.
            # scalar queue: 3 tiny weight DMAs only, so exps are never blocked
            # behind a long DMA_DIRECT2D descriptor-enqueue stall.
            # SWDGE: masks only, 5-buf runway, consumption-paced.
            q0a = half_dma(qT, 0)
            k0a = half_dma(kT, 0)
            q0b = half_dma(qT, 512)
            k0b = half_dma(kT, 512)
            v0a = half_dma(vT, 0)
            v0b = half_dma(vT, 512)
            kraws = {1: k_dma(1)}
            vraws = {1: v_dma(1)}
            kraws[2] = k_dma(2)
            vraws[2] = v_dma(2)
            kraws[3] = k_dma(3)
            vraws[3] = v_dma(3)
            qraws = {1: q_dma(1)}
            for g4 in range(4):
                mask_group(0, g4)

            def normalize(qc):
                outT_ps = outTs[qc]
                for qn in range(2):
                    c = slice(qn * 512, (qn + 1) * 512)
                    den_sb = lp.tile([1, 512], F32, tag="densb")
                    nc.vector.tensor_copy(den_sb[:, :], outT_ps[A:A + 1, c])
                    recip = lp.tile([1, 512], F32, tag="recip")
                    nc.vector.reciprocal_approx_fast(recip[:, :], den_sb[:, :])
                    rb_sb = lp.tile([A, 512], F32, tag="rbsb")
                    nc.gpsimd.partition_broadcast(rb_sb[:, :], recip[:, :], channels=A)
                    final = lp.tile([A, 512], F32, tag="final")
                    nc.vector.tensor_tensor(final[:, :], outT_ps[0:A, c], rb_sb[:, :], MULT)
                    nc.scalar.dma_start(out=out[:, qc * QW + qn * 512: qc * QW + (qn + 1) * 512], in_=final[:, :])

            proj_sub(wk_sb, k0a, kpT, 0)
            proj_sub(wq_sb, q0a, qpT, 0)
            proj_sub(wk_sb, k0b, kpT, 512)
            proj_sub(wq_sb, q0b, qpT, 512)
            vp_sub(v0a, 0)
            vp_sub(v0b, 4)
            outT0 = psO.tile([A + 1, QW], F32, tag="psO")
            outTs[0] = outT0

            # projections just-in-time: kp_g due at scores kt=8g (loop pos 8g),
            # vp_g due at tail kt=8g (pos 8g+2); DMAs get maximal lead time
            for kt in range(0, 7):
                attn_step(0, kt)
            kp_group(1, kraws[1])
            for kt in range(7, 9):
                attn_step(0, kt)
            vp_group(1, vraws[1])
            for kt in range(9, 15):
                attn_step(0, kt)
            kp_group(2, kraws[2])
            for kt in range(15, 17):
                attn_step(0, kt)
            vp_group(2, vraws[2])
            for kt in range(17, 23):
                attn_step(0, kt)
            kp_group(3, kraws[3])
            for kt in range(23, 25):
                attn_step(0, kt)
            vp_group(3, vraws[3])
            for kt in range(25, 27):
                attn_step(0, kt)
            qp_group(1, qraws[1])
            for kt in range(27, 32):
                attn_step(0, kt)
            # prefetch qc1's first masks ahead of the gpsimd-queued broadcast
            mask_group(1, 0)
            mask_group(1, 1)
            emit_tail(0, 30)
            emit_tail(0, 31)
            normalize(0)
            outT1 = psO.tile([A + 1, QW], F32, tag="psO")
            outTs[1] = outT1
            for kt in range(KT):
                attn_step(1, kt)
            emit_tail(1, 30)
            emit_tail(1, 31)
            normalize(1)

    nc.compile()
    return nc


def _get_nc():
    if "nc" not in _NC_CACHE:
        _NC_CACHE["nc"] = _build()
    return _NC_CACHE["nc"]


def _shard_inputs(q, k, v, mask, wq, wk, wv):
    """Full inputs -> per-core in_maps.  Host work is layout-only."""
    q = np.asarray(q, dtype=np.float32)
    k = np.asarray(k, dtype=np.float32)
    v = np.asarray(v, dtype=np.float32)
    wq = np.ascontiguousarray(np.asarray(wq, dtype=np.float32))
    wk = np.ascontiguousarray(np.asarray(wk, dtype=np.float32))
    wv = np.ascontiguousarray(np.asarray(wv, dtype=np.float32))
    mask = np.asarray(mask)
    if mask.dtype == np.bool_:
        maskbar = (~mask).view(np.uint8)
    else:
        maskbar = (mask == 0).view(np.uint8)
    in_maps = []
    for c in range(N_CORES):
        b, h = c // 2, c % 2
        sl = slice(h * QL, (h + 1) * QL)
        mbT = maskbar[b, sl, :].T  # [S keys, QL queries]
        mb_tiled = np.ascontiguousarray(
            mbT.reshape(8, 4, 128, QC, QW).transpose(3, 0, 2, 1, 4)
        )  # [QC, 8 groups, 128 p, 4 j, QW]
        in_maps.append({
            "qT": np.ascontiguousarray(q[b, sl, :].T),
            "kT": np.ascontiguousarray(k[b].T),
            "vT": np.ascontiguousarray(v[b].T),
            "mbT": mb_tiled,
            "mb16": mb_tiled[1].astype(np.float16),
            "wq": wq,
            "wk": wk,
            "wv": wv,
        })
    return in_maps


def _assemble_output(results):
    out = np.empty((B, S, A), dtype=np.float32)
    for c in range(N_CORES):
        b, h = c // 2, c % 2
        out[b, h * QL:(h + 1) * QL, :] = results[c]["out"].T
    return out


def run_sharded(in_maps, trace=False):
    """Compile (cached) + run the SPMD kernel on cores 0-7."""
    from concourse import bass_utils
    nc = _get_nc()
    return bass_utils.run_bass_kernel_spmd(
        nc, in_maps, core_ids=list(range(N_CORES)), trace=trace
    )


def kernel(q, k, v, mask, wq, wk, wv):
    """Full (unsharded) inputs -> full [B, S, A] float32 output."""
    in_maps = _shard_inputs(q, k, v, mask, wq, wk, wv)
    res = run_sharded(in_maps, trace=False)
    return _assemble_output(res.results)
